# revision 18
# baseline (speedup 1.0000x reference)
"""RBF Gram kernel K[i,j] = exp(-||x_i - y_j||^2) on 8 Trainium2 cores.

Sharding: rows of x (and of the output) split 8 ways; y replicated.
Per core: out[1024, 8192] = exp(2*(x@y^T) - x2[:,None] - y2[None,:]).

Per [128n x 2048m] group, a 3-engine pipeline (factorized exp):
    psum = x16^T y16                      TensorE, 4 fp16 matmuls (full K)
    t    = Exp(2*psum - x2_i - C)         ScalarE -> bf16   (C = 60)
    out  = t * eyg,  eyg_j = e^(C-y2_j)   VectorE bf16 2x mode
    DMA out (bf16), host upcasts to f32.

ScalarE's Exp (1 elem/lane/cycle) is the roofline; TensorE (1.7us/group
even at the cold 1.2 GHz clock) and VectorE (1.1us/group) hide under it.
Validated on the target regime: absmax error 2.8e-40 vs tolerance
1.45e-39 (5.1x margin); factor underflows only affect entries that are
below f32-denormal scale in the reference as well.
"""

import numpy as np
import ml_dtypes

import concourse.bass as bass
import concourse.bacc as bacc
import concourse.mybir as mybir
import concourse.tile as tile
from concourse.bass_utils import run_bass_kernel_spmd

F32 = mybir.dt.float32
F16 = mybir.dt.float16
BF16 = mybir.dt.bfloat16
BF = ml_dtypes.bfloat16

N = 8192          # rows of x / output
M = 8192          # rows of y / output cols
D = 128           # feature dim = contraction = partition dim
NCORES = 8
NS = N // NCORES  # 1024 output rows per core
NBLK = NS // 128  # 8 n-blocks per core
MGRP = 2048       # columns per PSUM group (4 banks)
NGRP = M // MGRP  # 4 groups
SUB = 512         # matmul moving size (1 PSUM bank fp32)
CSH = 60.0        # exponent shift between the two exp factors

_cached = {}


def _build_nc():
    nc = bacc.Bacc(None)

    yt = nc.dram_tensor("yt", [D, M], F16, kind="ExternalInput")
    xt = nc.dram_tensor("xt", [D, NS], F16, kind="ExternalInput")
    eyg = nc.dram_tensor("eyg", [128, M], BF16, kind="ExternalInput")
    nb = nc.dram_tensor("nb", [128, NBLK], F32, kind="ExternalInput")
    out = nc.dram_tensor("out", [NS, M], BF16, kind="ExternalOutput")

    with tile.TileContext(nc) as tc:
        with (
            tc.tile_pool(name="cst", bufs=1) as cst,
            tc.tile_pool(name="tp", bufs=4) as tp,
            tc.tile_pool(name="outp", bufs=6) as outp,
            tc.tile_pool(name="ps", bufs=2, space="PSUM") as ps,
        ):
            yt_t = cst.tile([D, M], F16, tag="yt")
            xt_t = cst.tile([D, NS], F16, tag="xt")
            eyg_t = cst.tile([128, M], BF16, tag="eyg")
            nb_t = cst.tile([128, NBLK], F32, tag="nb")
            wx_t = cst.tile([128, 640], F16, tag="wx")
            wo_t = cst.tile([128, 16], F32, tag="wo")
            # split input loads across BOTH HWDGE rings (Sync + Scalar) to
            # halve startup latency; issue order matches consumption order.
            # Scalar ring is only used before the ACT stream begins.
            nc.vector.memset(wx_t[:], 0.25)
            nc.sync.dma_start(yt_t[:, 0:SUB], yt[:, 0:SUB])
            nc.scalar.dma_start(xt_t[:, 0:128], xt[:, 0:128])
            nc.scalar.dma_start(nb_t[:], nb[:])
            nc.sync.dma_start(yt_t[:, SUB:1024], yt[:, SUB:1024])
            nc.sync.dma_start(yt_t[:, 1024:MGRP], yt[:, 1024:MGRP])
            # preload the Exp table set off the critical path
            nc.scalar.activation(
                wo_t[:], wx_t[:, 0:16], mybir.ActivationFunctionType.Exp)
            nc.scalar.dma_start(eyg_t[:, 0:MGRP], eyg[:, 0:MGRP])
            nc.sync.dma_start(xt_t[:, 128:NS], xt[:, 128:NS])
            for g in range(1, NGRP):
                sl = slice(g * MGRP, (g + 1) * MGRP)
                nc.sync.dma_start(yt_t[:, sl], yt[:, sl])
                nc.sync.dma_start(eyg_t[:, sl], eyg[:, sl])

            # g outer so early groups only touch y/eyg chunk 0 while the
            # remaining input chunks stream in behind compute
            for g in range(NGRP):
                for bi in range(NBLK):
                    xh_b = xt_t[:, bi * 128:(bi + 1) * 128]
                    first = g == 0 and bi == 0
                    last = g == NGRP - 1 and bi == NBLK - 1
                    p = ps.tile([128, MGRP], F32, tag="p")
                    t = tp.tile([128, MGRP], BF16, tag="t")
                    o = outp.tile([128, MGRP], BF16, tag="o")
                    # first group runs in quarters (ACT stream starts as soon
                    # as the first 512 columns of input land); last group in
                    # quarters (shortens the drain tail)
                    if first or last:
                        hs = (0, SUB, 2 * SUB, 3 * SUB, MGRP)
                    else:
                        hs = (0, MGRP)
                    for h0, h1 in zip(hs, hs[1:]):
                        for s in range(h0 // SUB, h1 // SUB):
                            m0 = g * MGRP + s * SUB
                            nc.tensor.matmul(
                                p[:, s * SUB:(s + 1) * SUB], xh_b,
                                yt_t[:, m0:m0 + SUB], start=True, stop=True)
                        nc.scalar.activation(
                            t[:, h0:h1], p[:, h0:h1],
                            mybir.ActivationFunctionType.Exp,
                            bias=nb_t[:, bi:bi + 1], scale=2.0)
                        if not last:
                            continue
                        nc.vector.tensor_mul(
                            o[:, h0:h1], t[:, h0:h1],
                            eyg_t[:, g * MGRP + h0:g * MGRP + h1])
                        nc.sync.dma_start(
                            out[bi * 128:(bi + 1) * 128,
                                g * MGRP + h0:g * MGRP + h1], o[:, h0:h1])
                    if last:
                        continue
                    nc.vector.tensor_mul(
                        o[:], t[:], eyg_t[:, g * MGRP:(g + 1) * MGRP])
                    nc.sync.dma_start(
                        out[bi * 128:(bi + 1) * 128, g * MGRP:(g + 1) * MGRP],
                        o[:])

    nc.finalize()
    return nc


def _prep_in_maps(x, y):
    x = np.ascontiguousarray(np.asarray(x, dtype=np.float32))
    y = np.ascontiguousarray(np.asarray(y, dtype=np.float32))
    assert x.shape == (N, D) and y.shape == (M, D)

    # host prep (O(N*D), trivial): transposes, fp16 casts, norms, exp(-y2)
    xt_f = x.T.astype(np.float16)                   # [D, N]
    yt_f = y.T.astype(np.float16)                   # [D, M]
    x2 = np.einsum("nd,nd->n", x, x, dtype=np.float64).astype(np.float32)
    y2 = np.einsum("md,md->m", y, y, dtype=np.float64).astype(np.float32)
    ey = np.exp((CSH - y2).astype(np.float32)).astype(BF)      # [M]
    eyg_v = np.ascontiguousarray(np.broadcast_to(ey, (128, M)))

    in_maps = []
    for c in range(NCORES):
        sl = slice(c * NS, (c + 1) * NS)
        nb_v = (-x2[sl] - np.float32(CSH)).reshape(NBLK, 128).T.copy()
        in_maps.append({
            "yt": np.ascontiguousarray(yt_f),
            "xt": np.ascontiguousarray(xt_f[:, sl]),
            "eyg": eyg_v,
            "nb": np.ascontiguousarray(nb_v),
        })
    return in_maps


def kernel(x, y):
    if "nc" not in _cached:
        _cached["nc"] = _build_nc()
    nc = _cached["nc"]
    in_maps = _prep_in_maps(x, y)
    res = run_bass_kernel_spmd(nc, in_maps, core_ids=list(range(NCORES)))
    return np.concatenate(
        [r["out"].astype(np.float32) for r in res.results], axis=0)


def run_traced(inputs):
    """Profiled run; returns BassKernelResults (exec_time_ns etc.)."""
    if "nc" not in _cached:
        _cached["nc"] = _build_nc()
    nc = _cached["nc"]
    in_maps = _prep_in_maps(**inputs)
    return run_bass_kernel_spmd(
        nc, in_maps, core_ids=list(range(NCORES)), trace=True)


# revision 19
# speedup vs baseline: 1.0084x; 1.0084x over previous
"""RBF Gram kernel K[i,j] = exp(-||x_i - y_j||^2) on 8 Trainium2 cores.

Sharding: rows of x (and of the output) split 8 ways; y replicated.
Per core: out[1024, 8192] = exp(2*(x@y^T) - x2[:,None] - y2[None,:]).

Per [128n x 2048m] group, a 3-engine pipeline (factorized exp):
    psum = x16^T y16                      TensorE, 4 fp16 matmuls (full K)
    t    = Exp(2*psum - x2_i - C)         ScalarE -> bf16   (C = 60)
    out  = t * eyg,  eyg_j = e^(C-y2_j)   VectorE bf16 2x mode
    DMA out (bf16), host upcasts to f32.

ScalarE's Exp (1 elem/lane/cycle) is the roofline; TensorE (1.7us/group
even at the cold 1.2 GHz clock) and VectorE (1.1us/group) hide under it.
Validated on the target regime: absmax error 2.8e-40 vs tolerance
1.45e-39 (5.1x margin); factor underflows only affect entries that are
below f32-denormal scale in the reference as well.
"""

import numpy as np
import ml_dtypes

import concourse.bass as bass
import concourse.bacc as bacc
import concourse.mybir as mybir
import concourse.tile as tile
from concourse.bass_utils import run_bass_kernel_spmd

F32 = mybir.dt.float32
F16 = mybir.dt.float16
BF16 = mybir.dt.bfloat16
BF = ml_dtypes.bfloat16

N = 8192          # rows of x / output
M = 8192          # rows of y / output cols
D = 128           # feature dim = contraction = partition dim
NCORES = 8
NS = N // NCORES  # 1024 output rows per core
NBLK = NS // 128  # 8 n-blocks per core
MGRP = 2048       # columns per PSUM group (4 banks)
NGRP = M // MGRP  # 4 groups
SUB = 512         # matmul moving size (1 PSUM bank fp32)
CSH = 60.0        # exponent shift between the two exp factors

_cached = {}


def _build_nc():
    nc = bacc.Bacc(None)

    yt = nc.dram_tensor("yt", [D, M], F16, kind="ExternalInput")
    xt = nc.dram_tensor("xt", [D, NS], F16, kind="ExternalInput")
    eyg = nc.dram_tensor("eyg", [128, M], BF16, kind="ExternalInput")
    nb = nc.dram_tensor("nb", [128, NBLK], F32, kind="ExternalInput")
    out = nc.dram_tensor("out", [NS, M], BF16, kind="ExternalOutput")

    with tile.TileContext(nc) as tc:
        with (
            tc.tile_pool(name="cst", bufs=1) as cst,
            tc.tile_pool(name="tp", bufs=4) as tp,
            tc.tile_pool(name="outp", bufs=6) as outp,
            tc.tile_pool(name="ps", bufs=2, space="PSUM") as ps,
        ):
            yt_t = cst.tile([D, M], F16, tag="yt")
            xt_t = cst.tile([D, NS], F16, tag="xt")
            eyg_t = cst.tile([128, M], BF16, tag="eyg")
            nb_t = cst.tile([128, NBLK], F32, tag="nb")
            wx_t = cst.tile([128, 640], F16, tag="wx")
            wo_t = cst.tile([128, 16], F32, tag="wo")
            # split input loads across BOTH HWDGE rings (Sync + Scalar) to
            # halve startup latency; issue order matches consumption order.
            # Scalar ring is only used before the ACT stream begins.
            nc.vector.memset(wx_t[:], 0.25)
            # HAM warm-up during the input-DMA wait: ~3.4us of dummy matmuls
            # ending before the first inputs land, so the PE enters the
            # steady loop at 2.4 GHz and stays under the ACT stream rate
            wp = ps.tile([128, MGRP], F32, tag="p")
            for w in range(8):
                nc.tensor.matmul(
                    wp[:, 0:SUB], wx_t[:, 0:128], wx_t[:, 128:640],
                    start=(w == 0), stop=(w == 7))
            nc.sync.dma_start(yt_t[:, 0:SUB], yt[:, 0:SUB])
            nc.scalar.dma_start(xt_t[:, 0:128], xt[:, 0:128])
            nc.scalar.dma_start(nb_t[:], nb[:])
            nc.sync.dma_start(yt_t[:, SUB:1024], yt[:, SUB:1024])
            nc.sync.dma_start(yt_t[:, 1024:MGRP], yt[:, 1024:MGRP])
            # preload the Exp table set off the critical path
            nc.scalar.activation(
                wo_t[:], wx_t[:, 0:16], mybir.ActivationFunctionType.Exp)
            nc.scalar.dma_start(eyg_t[:, 0:MGRP], eyg[:, 0:MGRP])
            nc.sync.dma_start(xt_t[:, 128:NS], xt[:, 128:NS])
            for g in range(1, NGRP):
                sl = slice(g * MGRP, (g + 1) * MGRP)
                nc.sync.dma_start(yt_t[:, sl], yt[:, sl])
                nc.sync.dma_start(eyg_t[:, sl], eyg[:, sl])

            # g outer so early groups only touch y/eyg chunk 0 while the
            # remaining input chunks stream in behind compute
            for g in range(NGRP):
                for bi in range(NBLK):
                    xh_b = xt_t[:, bi * 128:(bi + 1) * 128]
                    first = g == 0 and bi == 0
                    last = g == NGRP - 1 and bi == NBLK - 1
                    p = ps.tile([128, MGRP], F32, tag="p")
                    t = tp.tile([128, MGRP], BF16, tag="t")
                    o = outp.tile([128, MGRP], BF16, tag="o")
                    # first group runs in quarters (ACT stream starts as soon
                    # as the first 512 columns of input land); last group in
                    # quarters (shortens the drain tail)
                    if first or last:
                        hs = (0, SUB, 2 * SUB, 3 * SUB, MGRP)
                    else:
                        hs = (0, MGRP)
                    for h0, h1 in zip(hs, hs[1:]):
                        for s in range(h0 // SUB, h1 // SUB):
                            m0 = g * MGRP + s * SUB
                            nc.tensor.matmul(
                                p[:, s * SUB:(s + 1) * SUB], xh_b,
                                yt_t[:, m0:m0 + SUB], start=True, stop=True)
                        nc.scalar.activation(
                            t[:, h0:h1], p[:, h0:h1],
                            mybir.ActivationFunctionType.Exp,
                            bias=nb_t[:, bi:bi + 1], scale=2.0)
                        if not last:
                            continue
                        nc.vector.tensor_mul(
                            o[:, h0:h1], t[:, h0:h1],
                            eyg_t[:, g * MGRP + h0:g * MGRP + h1])
                        nc.sync.dma_start(
                            out[bi * 128:(bi + 1) * 128,
                                g * MGRP + h0:g * MGRP + h1], o[:, h0:h1])
                    if last:
                        continue
                    nc.vector.tensor_mul(
                        o[:], t[:], eyg_t[:, g * MGRP:(g + 1) * MGRP])
                    nc.sync.dma_start(
                        out[bi * 128:(bi + 1) * 128, g * MGRP:(g + 1) * MGRP],
                        o[:])

    nc.finalize()
    return nc


def _prep_in_maps(x, y):
    x = np.ascontiguousarray(np.asarray(x, dtype=np.float32))
    y = np.ascontiguousarray(np.asarray(y, dtype=np.float32))
    assert x.shape == (N, D) and y.shape == (M, D)

    # host prep (O(N*D), trivial): transposes, fp16 casts, norms, exp(-y2)
    xt_f = x.T.astype(np.float16)                   # [D, N]
    yt_f = y.T.astype(np.float16)                   # [D, M]
    x2 = np.einsum("nd,nd->n", x, x, dtype=np.float64).astype(np.float32)
    y2 = np.einsum("md,md->m", y, y, dtype=np.float64).astype(np.float32)
    ey = np.exp((CSH - y2).astype(np.float32)).astype(BF)      # [M]
    eyg_v = np.ascontiguousarray(np.broadcast_to(ey, (128, M)))

    in_maps = []
    for c in range(NCORES):
        sl = slice(c * NS, (c + 1) * NS)
        nb_v = (-x2[sl] - np.float32(CSH)).reshape(NBLK, 128).T.copy()
        in_maps.append({
            "yt": np.ascontiguousarray(yt_f),
            "xt": np.ascontiguousarray(xt_f[:, sl]),
            "eyg": eyg_v,
            "nb": np.ascontiguousarray(nb_v),
        })
    return in_maps


def kernel(x, y):
    if "nc" not in _cached:
        _cached["nc"] = _build_nc()
    nc = _cached["nc"]
    in_maps = _prep_in_maps(x, y)
    res = run_bass_kernel_spmd(nc, in_maps, core_ids=list(range(NCORES)))
    return np.concatenate(
        [r["out"].astype(np.float32) for r in res.results], axis=0)


def run_traced(inputs):
    """Profiled run; returns BassKernelResults (exec_time_ns etc.)."""
    if "nc" not in _cached:
        _cached["nc"] = _build_nc()
    nc = _cached["nc"]
    in_maps = _prep_in_maps(**inputs)
    return run_bass_kernel_spmd(
        nc, in_maps, core_ids=list(range(NCORES)), trace=True)


# revision 20
# speedup vs baseline: 1.0292x; 1.0206x over previous
"""RBF Gram kernel K[i,j] = exp(-||x_i - y_j||^2) on 8 Trainium2 cores.

Sharding: rows of x (and of the output) split 8 ways; y replicated.
Per core: out[1024, 8192] = exp(2*(x@y^T) - x2[:,None] - y2[None,:]).

Per [128n x 2048m] group, a 3-engine pipeline (factorized exp):
    psum = x16^T y16                      TensorE, 4 fp16 matmuls (full K)
    t    = Exp(2*psum - x2_i - C)         ScalarE -> bf16   (C = 60)
    out  = t * eyg,  eyg_j = e^(C-y2_j)   VectorE bf16 2x mode
    DMA out (bf16), host upcasts to f32.

ScalarE's Exp (1 elem/lane/cycle) is the roofline; TensorE (1.7us/group
even at the cold 1.2 GHz clock) and VectorE (1.1us/group) hide under it.
Validated on the target regime: absmax error 2.8e-40 vs tolerance
1.45e-39 (5.1x margin); factor underflows only affect entries that are
below f32-denormal scale in the reference as well.
"""

import numpy as np
import ml_dtypes

import concourse.bass as bass
import concourse.bacc as bacc
import concourse.mybir as mybir
import concourse.tile as tile
from concourse.bass_utils import run_bass_kernel_spmd

F32 = mybir.dt.float32
F16 = mybir.dt.float16
BF16 = mybir.dt.bfloat16
BF = ml_dtypes.bfloat16

N = 8192          # rows of x / output
M = 8192          # rows of y / output cols
D = 128           # feature dim = contraction = partition dim
NCORES = 8
NS = N // NCORES  # 1024 output rows per core
NBLK = NS // 128  # 8 n-blocks per core
MGRP = 2048       # columns per PSUM group (4 banks)
NGRP = M // MGRP  # 4 groups
SUB = 512         # matmul moving size (1 PSUM bank fp32)
CSH = 60.0        # exponent shift between the two exp factors

_cached = {}


def _build_nc():
    nc = bacc.Bacc(None)

    yt = nc.dram_tensor("yt", [D, M], F16, kind="ExternalInput")
    xt = nc.dram_tensor("xt", [D, NS], F16, kind="ExternalInput")
    eyg = nc.dram_tensor("eyg", [128, M], BF16, kind="ExternalInput")
    nb = nc.dram_tensor("nb", [128, NBLK], F32, kind="ExternalInput")
    out = nc.dram_tensor("out", [NS, M], BF16, kind="ExternalOutput")

    with tile.TileContext(nc) as tc:
        with (
            tc.tile_pool(name="cst", bufs=1) as cst,
            tc.tile_pool(name="tp", bufs=4) as tp,
            tc.tile_pool(name="outp", bufs=6) as outp,
            tc.tile_pool(name="ps", bufs=2, space="PSUM") as ps,
        ):
            yt_t = cst.tile([D, M], F16, tag="yt")
            xt_t = cst.tile([D, NS], F16, tag="xt")
            eyg_t = cst.tile([128, M], BF16, tag="eyg")
            nb_t = cst.tile([128, NBLK], F32, tag="nb")
            wx_t = cst.tile([128, 640], F16, tag="wx")
            wo_t = cst.tile([128, 16], F32, tag="wo")
            # issue order matches consumption order in the quartered start
            nc.sync.dma_start(yt_t[:, 0:SUB], yt[:, 0:SUB])
            nc.sync.dma_start(xt_t[:, 0:128], xt[:, 0:128])
            nc.sync.dma_start(nb_t[:], nb[:])
            nc.sync.dma_start(yt_t[:, SUB:1024], yt[:, SUB:1024])
            nc.sync.dma_start(yt_t[:, 1024:MGRP], yt[:, 1024:MGRP])
            nc.sync.dma_start(xt_t[:, 128:NS], xt[:, 128:NS])
            nc.sync.dma_start(eyg_t[:, 0:MGRP], eyg[:, 0:MGRP])
            for g in range(1, NGRP):
                sl = slice(g * MGRP, (g + 1) * MGRP)
                nc.sync.dma_start(yt_t[:, sl], yt[:, sl])
                nc.sync.dma_start(eyg_t[:, sl], eyg[:, sl])
            nc.vector.memset(wx_t[:], 0.25)
            # preload the Exp table set off the critical path
            nc.scalar.activation(
                wo_t[:], wx_t[:, 0:16], mybir.ActivationFunctionType.Exp)

            # g outer so early groups only touch y/eyg chunk 0 while the
            # remaining input chunks stream in behind compute
            for g in range(NGRP):
                for bi in range(NBLK):
                    xh_b = xt_t[:, bi * 128:(bi + 1) * 128]
                    first = g == 0 and bi == 0
                    last = g == NGRP - 1 and bi == NBLK - 1
                    p = ps.tile([128, MGRP], F32, tag="p")
                    t = tp.tile([128, MGRP], BF16, tag="t")
                    o = outp.tile([128, MGRP], BF16, tag="o")
                    # first group runs in quarters (ACT stream starts as soon
                    # as the first 512 columns of input land); last group in
                    # quarters (shortens the drain tail)
                    if first or last:
                        hs = (0, SUB, 2 * SUB, 3 * SUB, MGRP)
                    else:
                        hs = (0, MGRP)
                    for h0, h1 in zip(hs, hs[1:]):
                        for s in range(h0 // SUB, h1 // SUB):
                            m0 = g * MGRP + s * SUB
                            nc.tensor.matmul(
                                p[:, s * SUB:(s + 1) * SUB], xh_b,
                                yt_t[:, m0:m0 + SUB], start=True, stop=True)
                        nc.scalar.activation(
                            t[:, h0:h1], p[:, h0:h1],
                            mybir.ActivationFunctionType.Exp,
                            bias=nb_t[:, bi:bi + 1], scale=2.0)
                        if not last:
                            continue
                        nc.vector.tensor_mul(
                            o[:, h0:h1], t[:, h0:h1],
                            eyg_t[:, g * MGRP + h0:g * MGRP + h1])
                        nc.sync.dma_start(
                            out[bi * 128:(bi + 1) * 128,
                                g * MGRP + h0:g * MGRP + h1], o[:, h0:h1])
                    if last:
                        continue
                    nc.vector.tensor_mul(
                        o[:], t[:], eyg_t[:, g * MGRP:(g + 1) * MGRP])
                    nc.sync.dma_start(
                        out[bi * 128:(bi + 1) * 128, g * MGRP:(g + 1) * MGRP],
                        o[:])

    nc.finalize()
    return nc


def _prep_in_maps(x, y):
    x = np.ascontiguousarray(np.asarray(x, dtype=np.float32))
    y = np.ascontiguousarray(np.asarray(y, dtype=np.float32))
    assert x.shape == (N, D) and y.shape == (M, D)

    # host prep (O(N*D), trivial): transposes, fp16 casts, norms, exp(-y2)
    xt_f = x.T.astype(np.float16)                   # [D, N]
    yt_f = y.T.astype(np.float16)                   # [D, M]
    x2 = np.einsum("nd,nd->n", x, x, dtype=np.float64).astype(np.float32)
    y2 = np.einsum("md,md->m", y, y, dtype=np.float64).astype(np.float32)
    ey = np.exp((CSH - y2).astype(np.float32)).astype(BF)      # [M]
    eyg_v = np.ascontiguousarray(np.broadcast_to(ey, (128, M)))

    in_maps = []
    for c in range(NCORES):
        sl = slice(c * NS, (c + 1) * NS)
        nb_v = (-x2[sl] - np.float32(CSH)).reshape(NBLK, 128).T.copy()
        in_maps.append({
            "yt": np.ascontiguousarray(yt_f),
            "xt": np.ascontiguousarray(xt_f[:, sl]),
            "eyg": eyg_v,
            "nb": np.ascontiguousarray(nb_v),
        })
    return in_maps


def kernel(x, y):
    if "nc" not in _cached:
        _cached["nc"] = _build_nc()
    nc = _cached["nc"]
    in_maps = _prep_in_maps(x, y)
    res = run_bass_kernel_spmd(nc, in_maps, core_ids=list(range(NCORES)))
    return np.concatenate(
        [r["out"].astype(np.float32) for r in res.results], axis=0)


def run_traced(inputs):
    """Profiled run; returns BassKernelResults (exec_time_ns etc.)."""
    if "nc" not in _cached:
        _cached["nc"] = _build_nc()
    nc = _cached["nc"]
    in_maps = _prep_in_maps(**inputs)
    return run_bass_kernel_spmd(
        nc, in_maps, core_ids=list(range(NCORES)), trace=True)


# revision 22
# speedup vs baseline: 1.0450x; 1.0153x over previous
"""RBF Gram kernel K[i,j] = exp(-||x_i - y_j||^2) on 8 Trainium2 cores.

Sharding: rows of x (and of the output) split 8 ways; y replicated.
Per core: out[1024, 8192] = exp(2*(x@y^T) - x2[:,None] - y2[None,:]).

Per [128n x 2048m] group, a 3-engine pipeline (factorized exp):
    psum = x16^T y16                      TensorE, 4 fp16 matmuls (full K)
    t    = Exp(2*psum - x2_i - C)         ScalarE -> bf16   (C = 60)
    out  = t * eyg,  eyg_j = e^(C-y2_j)   VectorE bf16 2x mode
    DMA out (bf16), host upcasts to f32.

ScalarE's Exp (1 elem/lane/cycle) is the roofline; TensorE (1.7us/group
even at the cold 1.2 GHz clock) and VectorE (1.1us/group) hide under it.
Validated on the target regime: absmax error 2.8e-40 vs tolerance
1.45e-39 (5.1x margin); factor underflows only affect entries that are
below f32-denormal scale in the reference as well.
"""

import numpy as np
import ml_dtypes

import concourse.bass as bass
import concourse.bacc as bacc
import concourse.mybir as mybir
import concourse.tile as tile
from concourse.bass_utils import run_bass_kernel_spmd

F32 = mybir.dt.float32
F16 = mybir.dt.float16
BF16 = mybir.dt.bfloat16
BF = ml_dtypes.bfloat16

N = 8192          # rows of x / output
M = 8192          # rows of y / output cols
D = 128           # feature dim = contraction = partition dim
NCORES = 8
NS = N // NCORES  # 1024 output rows per core
NBLK = NS // 128  # 8 n-blocks per core
MGRP = 2048       # columns per PSUM group (4 banks)
NGRP = M // MGRP  # 4 groups
SUB = 512         # matmul moving size (1 PSUM bank fp32)
CSH = 60.0        # exponent shift between the two exp factors

_cached = {}


def _build_nc():
    nc = bacc.Bacc(None)

    yt = nc.dram_tensor("yt", [D, M], F16, kind="ExternalInput")
    xt = nc.dram_tensor("xt", [D, NS], F16, kind="ExternalInput")
    eyg = nc.dram_tensor("eyg", [128, M], BF16, kind="ExternalInput")
    nb = nc.dram_tensor("nb", [128, NBLK], F32, kind="ExternalInput")
    out = nc.dram_tensor("out", [NS, M], BF16, kind="ExternalOutput")

    with tile.TileContext(nc) as tc:
        with (
            tc.tile_pool(name="cst", bufs=1) as cst,
            tc.tile_pool(name="tp", bufs=4) as tp,
            tc.tile_pool(name="outp", bufs=6) as outp,
            tc.tile_pool(name="ps", bufs=2, space="PSUM") as ps,
        ):
            yt_t = cst.tile([D, M], F16, tag="yt")
            xt_t = cst.tile([D, NS], F16, tag="xt")
            eyg_t = cst.tile([128, M], BF16, tag="eyg")
            nb_t = cst.tile([128, NBLK], F32, tag="nb")
            wx_t = cst.tile([128, 640], F16, tag="wx")
            wo_t = cst.tile([128, 16], F32, tag="wo")
            # HAM warm-up abutting the stream: ~5.5us of dummy matmuls that
            # end right as the first inputs land, leaving no PE-idle window
            # for the MID monitor to catch before the steady loop begins
            nc.vector.memset(wx_t[:], 0.25)
            wp = ps.tile([128, MGRP], F32, tag="p")
            for w in range(13):
                nc.tensor.matmul(
                    wp[:, 0:SUB], wx_t[:, 0:128], wx_t[:, 128:640],
                    start=(w == 0), stop=(w == 12))
            # issue order matches consumption order in the quartered start
            nc.sync.dma_start(yt_t[:, 0:SUB], yt[:, 0:SUB])
            nc.sync.dma_start(xt_t[:, 0:128], xt[:, 0:128])
            nc.sync.dma_start(nb_t[:], nb[:])
            nc.sync.dma_start(yt_t[:, SUB:1024], yt[:, SUB:1024])
            nc.sync.dma_start(yt_t[:, 1024:MGRP], yt[:, 1024:MGRP])
            nc.sync.dma_start(xt_t[:, 128:NS], xt[:, 128:NS])
            nc.sync.dma_start(eyg_t[:, 0:MGRP], eyg[:, 0:MGRP])
            for g in range(1, NGRP):
                sl = slice(g * MGRP, (g + 1) * MGRP)
                nc.sync.dma_start(yt_t[:, sl], yt[:, sl])
                nc.sync.dma_start(eyg_t[:, sl], eyg[:, sl])
            # preload the Exp table set off the critical path
            nc.scalar.activation(
                wo_t[:], wx_t[:, 0:16], mybir.ActivationFunctionType.Exp)

            # g outer so early groups only touch y/eyg chunk 0 while the
            # remaining input chunks stream in behind compute
            for g in range(NGRP):
                for bi in range(NBLK):
                    xh_b = xt_t[:, bi * 128:(bi + 1) * 128]
                    first = g == 0 and bi == 0
                    last = g == NGRP - 1 and bi == NBLK - 1
                    p = ps.tile([128, MGRP], F32, tag="p")
                    t = tp.tile([128, MGRP], BF16, tag="t")
                    o = outp.tile([128, MGRP], BF16, tag="o")
                    # first group runs in quarters (ACT stream starts as soon
                    # as the first 512 columns of input land); last group in
                    # quarters (shortens the drain tail)
                    if first or last:
                        hs = (0, SUB, 2 * SUB, 3 * SUB, MGRP)
                    else:
                        hs = (0, MGRP)
                    for h0, h1 in zip(hs, hs[1:]):
                        for s in range(h0 // SUB, h1 // SUB):
                            m0 = g * MGRP + s * SUB
                            nc.tensor.matmul(
                                p[:, s * SUB:(s + 1) * SUB], xh_b,
                                yt_t[:, m0:m0 + SUB], start=True, stop=True)
                        nc.scalar.activation(
                            t[:, h0:h1], p[:, h0:h1],
                            mybir.ActivationFunctionType.Exp,
                            bias=nb_t[:, bi:bi + 1], scale=2.0)
                        if not last:
                            continue
                        nc.vector.tensor_mul(
                            o[:, h0:h1], t[:, h0:h1],
                            eyg_t[:, g * MGRP + h0:g * MGRP + h1])
                        nc.sync.dma_start(
                            out[bi * 128:(bi + 1) * 128,
                                g * MGRP + h0:g * MGRP + h1], o[:, h0:h1])
                    if last:
                        continue
                    nc.vector.tensor_mul(
                        o[:], t[:], eyg_t[:, g * MGRP:(g + 1) * MGRP])
                    nc.sync.dma_start(
                        out[bi * 128:(bi + 1) * 128, g * MGRP:(g + 1) * MGRP],
                        o[:])

    nc.finalize()
    return nc


def _prep_in_maps(x, y):
    x = np.ascontiguousarray(np.asarray(x, dtype=np.float32))
    y = np.ascontiguousarray(np.asarray(y, dtype=np.float32))
    assert x.shape == (N, D) and y.shape == (M, D)

    # host prep (O(N*D), trivial): transposes, fp16 casts, norms, exp(-y2)
    xt_f = x.T.astype(np.float16)                   # [D, N]
    yt_f = y.T.astype(np.float16)                   # [D, M]
    x2 = np.einsum("nd,nd->n", x, x, dtype=np.float64).astype(np.float32)
    y2 = np.einsum("md,md->m", y, y, dtype=np.float64).astype(np.float32)
    ey = np.exp((CSH - y2).astype(np.float32)).astype(BF)      # [M]
    eyg_v = np.ascontiguousarray(np.broadcast_to(ey, (128, M)))

    in_maps = []
    for c in range(NCORES):
        sl = slice(c * NS, (c + 1) * NS)
        nb_v = (-x2[sl] - np.float32(CSH)).reshape(NBLK, 128).T.copy()
        in_maps.append({
            "yt": np.ascontiguousarray(yt_f),
            "xt": np.ascontiguousarray(xt_f[:, sl]),
            "eyg": eyg_v,
            "nb": np.ascontiguousarray(nb_v),
        })
    return in_maps


def kernel(x, y):
    if "nc" not in _cached:
        _cached["nc"] = _build_nc()
    nc = _cached["nc"]
    in_maps = _prep_in_maps(x, y)
    res = run_bass_kernel_spmd(nc, in_maps, core_ids=list(range(NCORES)))
    return np.concatenate(
        [r["out"].astype(np.float32) for r in res.results], axis=0)


def run_traced(inputs):
    """Profiled run; returns BassKernelResults (exec_time_ns etc.)."""
    if "nc" not in _cached:
        _cached["nc"] = _build_nc()
    nc = _cached["nc"]
    in_maps = _prep_in_maps(**inputs)
    return run_bass_kernel_spmd(
        nc, in_maps, core_ids=list(range(NCORES)), trace=True)
